# revision 1
# baseline (speedup 1.0000x reference)
"""Trainium2 Bass kernel for a 4-layer MLP over N=100000 rows (DHGCN forward).

Reference computation (the graph edge_index `g` is dead):
    h = relu(x @ W0 + b0); h = relu(h @ W1 + b1)
    h = relu(h @ W2 + b2); h = relu(h @ W3 + b3)
with x [100000, 3000], W0 [3000,512], W1/W2 [512,512], W3 [512,20].

Strategy: data-parallel over rows across 8 NeuronCores (weights replicated).
On host, x is transposed to feature-major (xT) and the feature dim padded
3000 -> 3072 = 24*128 so activations live on-chip as [feat_part, row] tiles;
every matmul is then out[M=out_feat_chunk, N=rows] = W_tile.T @ hT_tile with
natural-layout weights and no on-device transposes.

Matmul operands use dtype float32r (same bits as fp32): plain-fp32 matmul
runs at 1/4 PE rate on TRN2, while f32r streams 1 row/cycle (full rate) at
free dim >= 256, at ~tf32 effective precision (measured rel err ~3e-4 for
the full 4-layer chain). The BIR verifier requires every producer feeding an
f32r matmul to emit f32r itself, so the x/weight DRAM tensors, their SBUF
tiles, and the relu-activation outputs (h tiles) are all declared f32r;
PSUM accumulation stays fp32 and the final output is fp32.
"""

import numpy as np

import concourse.bacc as bacc
import concourse.mybir as mybir
import concourse.tile as tile
from concourse.bass import ts
from concourse.bass_utils import run_bass_kernel_spmd

F32 = mybir.dt.float32
F32R = mybir.dt.float32r
RELU = mybir.ActivationFunctionType.Relu

N_CORES = 8
N_ROWS = 100000
ROWS_PER_CORE = N_ROWS // N_CORES  # 12500
R = 500                            # row-block (PSUM free dim <= 512)
N_BLK = ROWS_PER_CORE // R         # 25
IN_DIM = 3000
K0 = 3072                          # padded in_dim = 24*128
KT0 = K0 // 128                    # 24 K-tiles for layer 0
H = 512
KT = H // 128                      # 4 K-tiles for layers 1-3
M_CH = H // 128                    # 4 output chunks of 128 for layers 0-2
LAT = 20


def build_program():
    nc = bacc.Bacc("TRN2", target_bir_lowering=False, debug=False)

    xT = nc.dram_tensor("xT", [K0, ROWS_PER_CORE], F32R, kind="ExternalInput")
    w0 = nc.dram_tensor("w0", [K0, H], F32R, kind="ExternalInput")
    w1 = nc.dram_tensor("w1", [H, H], F32R, kind="ExternalInput")
    w2 = nc.dram_tensor("w2", [H, H], F32R, kind="ExternalInput")
    w3 = nc.dram_tensor("w3", [H, LAT], F32R, kind="ExternalInput")
    b0 = nc.dram_tensor("b0", [H], F32, kind="ExternalInput")
    b1 = nc.dram_tensor("b1", [H], F32, kind="ExternalInput")
    b2 = nc.dram_tensor("b2", [H], F32, kind="ExternalInput")
    b3 = nc.dram_tensor("b3", [LAT], F32, kind="ExternalInput")
    outT = nc.dram_tensor("outT", [LAT, ROWS_PER_CORE], F32, kind="ExternalOutput")

    xr = xT.rearrange("(ko p) r -> p ko r", p=128)    # [128, 24, 12500]
    w0r = w0.rearrange("(ko p) f -> p ko f", p=128)   # [128, 24, 512]
    w1r = w1.rearrange("(ko p) f -> p ko f", p=128)   # [128, 4, 512]
    w2r = w2.rearrange("(ko p) f -> p ko f", p=128)
    w3r = w3.rearrange("(ko p) f -> p ko f", p=128)   # [128, 4, 20]
    b0r = b0.rearrange("(m p) -> p m", p=128)         # [128, 4]
    b1r = b1.rearrange("(m p) -> p m", p=128)
    b2r = b2.rearrange("(m p) -> p m", p=128)
    b3r = b3.rearrange("(m p) -> p m", p=LAT)         # [20, 1]

    with tile.TileContext(nc) as tc:
        with (
            tc.tile_pool(name="const", bufs=1) as const,
            tc.tile_pool(name="xin", bufs=2) as xin,
            tc.tile_pool(name="hbuf", bufs=1) as hbuf,
            tc.tile_pool(name="obuf", bufs=2) as obuf,
            tc.tile_pool(name="psA", bufs=4, space="PSUM") as psA,
            tc.tile_pool(name="psB", bufs=2, space="PSUM") as psB,
        ):
            w0_sb = const.tile([128, KT0, H], F32R, tag="w0")
            w1_sb = const.tile([128, KT, H], F32R, tag="w1")
            w2_sb = const.tile([128, KT, H], F32R, tag="w2")
            w3_sb = const.tile([128, KT, LAT], F32R, tag="w3")
            b0_sb = const.tile([128, M_CH], F32, tag="b0")
            b1_sb = const.tile([128, M_CH], F32, tag="b1")
            b2_sb = const.tile([128, M_CH], F32, tag="b2")
            b3_sb = const.tile([LAT, 1], F32, tag="b3")
            for ko_w in range(KT0):
                nc.sync.dma_start(w0_sb[:, ko_w, :], w0r[:, ko_w, :])
            nc.sync.dma_start(w1_sb[:], w1r[:])
            nc.sync.dma_start(w2_sb[:], w2r[:])
            nc.sync.dma_start(w3_sb[:], w3r[:])
            nc.sync.dma_start(b0_sb[:], b0r[:])
            nc.sync.dma_start(b1_sb[:], b1r[:])
            nc.sync.dma_start(b2_sb[:], b2r[:])
            nc.sync.dma_start(b3_sb[:], b3r[:])

            for j in range(N_BLK):
                x_t = xin.tile([128, KT0, R], F32R, tag="x")
                if j == 0:
                    # Split w0 and block-0 x into per-K-tile DMAs so the first
                    # matmul group starts once its first tiles land instead of
                    # waiting out the full 12 MB startup load (-31 us model).
                    for ko in range(KT0):
                        nc.sync.dma_start(x_t[:, ko, :], xr[:, ko, ts(j, R)])
                else:
                    nc.sync.dma_start(x_t[:], xr[:, :, ts(j, R)])

                h1 = hbuf.tile([128, KT, R], F32R, tag="h1")
                for m in range(M_CH):
                    ps = psA.tile([128, R], F32, tag="ps")
                    for ko in range(KT0):
                        nc.tensor.matmul(
                            ps[:],
                            w0_sb[:, ko, ts(m, 128)],
                            x_t[:, ko, :],
                            start=(ko == 0),
                            stop=(ko == KT0 - 1),
                        )
                    nc.scalar.activation(
                        h1[:, m, :], ps[:], RELU, bias=b0_sb[:, m : m + 1]
                    )

                h2 = hbuf.tile([128, KT, R], F32R, tag="h2")
                for m in range(M_CH):
                    ps = psA.tile([128, R], F32, tag="ps")
                    for ko in range(KT):
                        nc.tensor.matmul(
                            ps[:],
                            w1_sb[:, ko, ts(m, 128)],
                            h1[:, ko, :],
                            start=(ko == 0),
                            stop=(ko == KT - 1),
                        )
                    nc.scalar.activation(
                        h2[:, m, :], ps[:], RELU, bias=b1_sb[:, m : m + 1]
                    )

                h3 = hbuf.tile([128, KT, R], F32R, tag="h3")
                for m in range(M_CH):
                    ps = psA.tile([128, R], F32, tag="ps")
                    for ko in range(KT):
                        nc.tensor.matmul(
                            ps[:],
                            w2_sb[:, ko, ts(m, 128)],
                            h2[:, ko, :],
                            start=(ko == 0),
                            stop=(ko == KT - 1),
                        )
                    nc.scalar.activation(
                        h3[:, m, :], ps[:], RELU, bias=b2_sb[:, m : m + 1]
                    )

                ps3 = psB.tile([LAT, R], F32, tag="ps3")
                for ko in range(KT):
                    nc.tensor.matmul(
                        ps3[:],
                        w3_sb[:, ko, :],
                        h3[:, ko, :],
                        start=(ko == 0),
                        stop=(ko == KT - 1),
                    )
                o_t = obuf.tile([LAT, R], F32, tag="o")
                nc.scalar.activation(o_t[:], ps3[:], RELU, bias=b3_sb[:])
                nc.sync.dma_start(outT[:, ts(j, R)], o_t[:])

    nc.compile()
    return nc


_NC = None


def _get_nc():
    global _NC
    if _NC is None:
        _NC = build_program()
    return _NC


def make_in_maps(inputs, W0, b0, W1, b1, W2, b2, W3, b3):
    """Host-side sharding: pad features to K0, transpose x to feature-major,
    slice rows across cores; weights replicated.

    Builds one [N_CORES*K0, ROWS_PER_CORE] buffer so each core's xT is a
    contiguous view (bass2jax's per-core np.asarray is then copy-free), and
    uses a row-blocked transpose (cache-friendlier than one big x.T assign).
    """
    x = np.asarray(inputs, dtype=np.float32)
    xT_cat = np.empty((N_CORES * K0, ROWS_PER_CORE), dtype=np.float32)
    RB = 3125  # transpose block: RB rows at a time
    for c in range(N_CORES):
        base = c * K0
        r0 = c * ROWS_PER_CORE
        for rb in range(0, ROWS_PER_CORE, RB):
            xT_cat[base : base + IN_DIM, rb : rb + RB] = x[
                r0 + rb : r0 + rb + RB
            ].T
        xT_cat[base + IN_DIM : base + K0] = 0.0
    w0p = np.zeros((K0, H), dtype=np.float32)
    w0p[:IN_DIM] = np.asarray(W0, dtype=np.float32)
    common = {
        "w0": w0p,
        "w1": np.ascontiguousarray(W1, dtype=np.float32),
        "w2": np.ascontiguousarray(W2, dtype=np.float32),
        "w3": np.ascontiguousarray(W3, dtype=np.float32),
        "b0": np.ascontiguousarray(b0, dtype=np.float32),
        "b1": np.ascontiguousarray(b1, dtype=np.float32),
        "b2": np.ascontiguousarray(b2, dtype=np.float32),
        "b3": np.ascontiguousarray(b3, dtype=np.float32),
    }
    in_maps = []
    for c in range(N_CORES):
        in_maps.append({"xT": xT_cat[c * K0 : (c + 1) * K0], **common})
    return in_maps


def kernel(inputs, g, W0, b0, W1, b1, W2, b2, W3, b3):
    nc = _get_nc()
    in_maps = make_in_maps(inputs, W0, b0, W1, b1, W2, b2, W3, b3)
    res = run_bass_kernel_spmd(nc, in_maps, core_ids=list(range(N_CORES)))
    out = np.empty((N_ROWS, LAT), dtype=np.float32)
    for c, r in enumerate(res.results):
        out[c * ROWS_PER_CORE : (c + 1) * ROWS_PER_CORE] = r["outT"].T
    return out



# revision 3
# speedup vs baseline: 2.0622x; 2.0622x over previous
"""Trainium2 Bass kernel for a 4-layer MLP over N=100000 rows (DHGCN forward).

Reference computation (the graph edge_index `g` is dead):
    h = relu(x @ W0 + b0); h = relu(h @ W1 + b1)
    h = relu(h @ W2 + b2); h = relu(h @ W3 + b3)
with x [100000, 3000], W0 [3000,512], W1/W2 [512,512], W3 [512,20].

Strategy: data-parallel over rows across 8 NeuronCores (weights replicated).
On host, x is cast to bf16 and permuted to [128 part, block, ktile, row] so
every row-block DMA is 128 fully-contiguous 24KB descriptors; every matmul is
out[M=out_feat_chunk, N=rows] = W_tile.T @ hT_tile with natural-layout
weights and no on-device transposes.

Matmul operands are bfloat16 (PSUM accumulation stays fp32, biases and the
final output are fp32). bf16 streams 1 column/cycle on the PE like f32r, but
its weight loads take the fast-weight-load path the engine queue hoists ahead
of in-flight matmuls, where f32r's 4-byte self-loading InstMatmult serializes
~128+ weight-load cycles onto every matmul (the cost model doesn't model
LDWEIGHTS; HW-measured marginal exec was 944us vs the 690us PE floor with
f32r). bf16 also halves HBM traffic. Measured end-to-end rel err ~5e-3
against the fp32 reference (gate 2e-2).

Row blocks are emitted in software-pipelined pairs
(L0(a) L0(b) L1(a) L1(b) L2(a) L2(b) L3(a) L3(b)): every matmul group has a
whole other group between it and the activations that produce its input, so
the PE never stalls on the PSUM-drain + activation latency at layer
transitions.
"""

import numpy as np
import ml_dtypes

import concourse.bacc as bacc
import concourse.mybir as mybir
import concourse.tile as tile
from concourse.bass import ts
from concourse.bass_utils import run_bass_kernel_spmd

F32 = mybir.dt.float32
BF16 = mybir.dt.bfloat16
BF = ml_dtypes.bfloat16
RELU = mybir.ActivationFunctionType.Relu

N_CORES = 8
N_ROWS = 100000
ROWS_PER_CORE = N_ROWS // N_CORES  # 12500
R = 500                            # row-block (PSUM free dim <= 512)
N_BLK = ROWS_PER_CORE // R         # 25
IN_DIM = 3000
K0 = 3072                          # padded in_dim = 24*128
KT0 = K0 // 128                    # 24 K-tiles for layer 0
KT0_FULL = IN_DIM // 128           # 23 full K-tiles before the ragged one
KT0_REM = IN_DIM - KT0_FULL * 128  # 56 valid partitions in K-tile 23
H = 512
KT = H // 128                      # 4 K-tiles for layers 1-3
M_CH = H // 128                    # 4 output chunks of 128 for layers 0-2
LAT = 20


def build_program():
    nc = bacc.Bacc("TRN2", target_bir_lowering=False, debug=False)

    xb = nc.dram_tensor("xb", [128, N_BLK, KT0, R], BF16, kind="ExternalInput")
    w0 = nc.dram_tensor("w0", [K0, H], BF16, kind="ExternalInput")
    w1 = nc.dram_tensor("w1", [H, H], BF16, kind="ExternalInput")
    w2 = nc.dram_tensor("w2", [H, H], BF16, kind="ExternalInput")
    w3 = nc.dram_tensor("w3", [H, LAT], BF16, kind="ExternalInput")
    b0 = nc.dram_tensor("b0", [H], F32, kind="ExternalInput")
    b1 = nc.dram_tensor("b1", [H], F32, kind="ExternalInput")
    b2 = nc.dram_tensor("b2", [H], F32, kind="ExternalInput")
    b3 = nc.dram_tensor("b3", [LAT], F32, kind="ExternalInput")
    outT = nc.dram_tensor("outT", [LAT, ROWS_PER_CORE], F32, kind="ExternalOutput")

    w0r = w0.rearrange("(ko p) f -> p ko f", p=128)   # [128, 24, 512]
    w1r = w1.rearrange("(ko p) f -> p ko f", p=128)   # [128, 4, 512]
    w2r = w2.rearrange("(ko p) f -> p ko f", p=128)
    w3r = w3.rearrange("(ko p) f -> p ko f", p=128)   # [128, 4, 20]
    b0r = b0.rearrange("(m p) -> p m", p=128)         # [128, 4]
    b1r = b1.rearrange("(m p) -> p m", p=128)
    b2r = b2.rearrange("(m p) -> p m", p=128)
    b3r = b3.rearrange("(m p) -> p m", p=LAT)         # [20, 1]

    with tile.TileContext(nc) as tc:
        with (
            tc.tile_pool(name="const", bufs=1) as const,
            tc.tile_pool(name="xin", bufs=4) as xin,
            tc.tile_pool(name="hbuf", bufs=1) as hbuf,
            tc.tile_pool(name="obuf", bufs=2) as obuf,
            tc.tile_pool(name="psA", bufs=6, space="PSUM") as psA,
            tc.tile_pool(name="psB", bufs=2, space="PSUM") as psB,
        ):
            w0_sb = const.tile([128, KT0, H], BF16, tag="w0")
            w1_sb = const.tile([128, KT, H], BF16, tag="w1")
            w2_sb = const.tile([128, KT, H], BF16, tag="w2")
            w3_sb = const.tile([128, KT, LAT], BF16, tag="w3")
            b0_sb = const.tile([128, M_CH], F32, tag="b0")
            b1_sb = const.tile([128, M_CH], F32, tag="b1")
            b2_sb = const.tile([128, M_CH], F32, tag="b2")
            b3_sb = const.tile([LAT, 1], F32, tag="b3")

            x_ts = {}

            # Startup: interleave w0 K-tile loads with block-0 x chunks so the
            # first accumulation group starts as soon as its first tiles land,
            # then stream the rest of the constants.
            x_ts[0] = xin.tile([128, KT0, R], BF16, tag="x", name="x0")
            CH = 6
            for c0 in range(0, KT0, CH):
                for ko in range(c0, c0 + CH):
                    nc.sync.dma_start(w0_sb[:, ko, :], w0r[:, ko, :])
                nc.sync.dma_start(
                    x_ts[0][:, c0 : c0 + CH, :], xb[:, 0, c0 : c0 + CH, :]
                )
            nc.sync.dma_start(b0_sb[:], b0r[:])
            x_ts[1] = xin.tile([128, KT0, R], BF16, tag="x", name="x1")
            nc.sync.dma_start(x_ts[1][:], xb[:, 1])
            nc.sync.dma_start(w1_sb[:], w1r[:])
            nc.sync.dma_start(b1_sb[:], b1r[:])
            nc.sync.dma_start(w2_sb[:], w2r[:])
            nc.sync.dma_start(b2_sb[:], b2r[:])
            nc.sync.dma_start(w3_sb[:], w3r[:])
            nc.sync.dma_start(b3_sb[:], b3r[:])

            def emit_l0(x_t, h1):
                for m in range(M_CH):
                    ps = psA.tile([128, R], F32, tag="ps")
                    for ko in range(KT0):
                        nc.tensor.matmul(
                            ps[:],
                            w0_sb[:, ko, ts(m, 128)],
                            x_t[:, ko, :],
                            start=(ko == 0),
                            stop=(ko == KT0 - 1),
                        )
                    nc.scalar.activation(
                        h1[:, m, :], ps[:], RELU, bias=b0_sb[:, m : m + 1]
                    )

            def emit_mid(w_sb, b_sb, hin, hout):
                for m in range(M_CH):
                    ps = psA.tile([128, R], F32, tag="ps")
                    for ko in range(KT):
                        nc.tensor.matmul(
                            ps[:],
                            w_sb[:, ko, ts(m, 128)],
                            hin[:, ko, :],
                            start=(ko == 0),
                            stop=(ko == KT - 1),
                        )
                    nc.scalar.activation(
                        hout[:, m, :], ps[:], RELU, bias=b_sb[:, m : m + 1]
                    )

            def emit_l3(j, h3):
                ps3 = psB.tile([LAT, R], F32, tag="ps3")
                for ko in range(KT):
                    nc.tensor.matmul(
                        ps3[:],
                        w3_sb[:, ko, :],
                        h3[:, ko, :],
                        start=(ko == 0),
                        stop=(ko == KT - 1),
                    )
                o_t = obuf.tile([LAT, R], F32, tag="o")
                nc.scalar.activation(o_t[:], ps3[:], RELU, bias=b3_sb[:])
                nc.sync.dma_start(outT[:, ts(j, R)], o_t[:])

            for a in range(0, N_BLK, 2):
                b = a + 1 if a + 1 < N_BLK else None
                h1a = hbuf.tile([128, KT, R], BF16, tag="h1a")
                emit_l0(x_ts[a], h1a)
                if b is not None:
                    h1b = hbuf.tile([128, KT, R], BF16, tag="h1b")
                    emit_l0(x_ts[b], h1b)
                # prefetch the next pair while this pair computes
                for nxt in (a + 2, a + 3):
                    if nxt < N_BLK:
                        x_ts[nxt] = xin.tile([128, KT0, R], BF16, tag="x", name=f"x{nxt}")
                        nc.sync.dma_start(x_ts[nxt][:], xb[:, nxt])
                h2a = hbuf.tile([128, KT, R], BF16, tag="h2a")
                emit_mid(w1_sb, b1_sb, h1a, h2a)
                if b is not None:
                    h2b = hbuf.tile([128, KT, R], BF16, tag="h2b")
                    emit_mid(w1_sb, b1_sb, h1b, h2b)
                h3a = hbuf.tile([128, KT, R], BF16, tag="h3a")
                emit_mid(w2_sb, b2_sb, h2a, h3a)
                if b is not None:
                    h3b = hbuf.tile([128, KT, R], BF16, tag="h3b")
                    emit_mid(w2_sb, b2_sb, h2b, h3b)
                emit_l3(a, h3a)
                if b is not None:
                    emit_l3(b, h3b)

    nc.compile()
    return nc


_NC = None


def _get_nc():
    global _NC
    if _NC is None:
        _NC = build_program()
    return _NC


def make_in_maps(inputs, W0, b0, W1, b1, W2, b2, W3, b3):
    """Host-side sharding: cast x to bf16 and permute per core to
    [128 part, N_BLK, KT0, R] with feature-tile padding zeroed, so each
    row-block DMA is 128 contiguous 24KB partition lines. Weights replicated
    (bf16); biases fp32.
    """
    x = np.asarray(inputs, dtype=np.float32)
    xb_cat = np.empty((N_CORES * 128, N_BLK, KT0, R), dtype=BF)
    for c in range(N_CORES):
        xc = x[c * ROWS_PER_CORE : (c + 1) * ROWS_PER_CORE].astype(BF)
        src = xc.reshape(N_BLK, R, IN_DIM)
        v = xb_cat[c * 128 : (c + 1) * 128]  # [128, N_BLK, KT0, R] view
        full = src[:, :, : KT0_FULL * 128].reshape(N_BLK, R, KT0_FULL, 128)
        v[:, :, :KT0_FULL, :] = full.transpose(3, 0, 2, 1)
        part = src[:, :, KT0_FULL * 128 :]  # [N_BLK, R, KT0_REM]
        v[:KT0_REM, :, KT0_FULL, :] = part.transpose(2, 0, 1)
        v[KT0_REM:, :, KT0_FULL, :] = 0
    w0p = np.zeros((K0, H), dtype=BF)
    w0p[:IN_DIM] = np.asarray(W0, dtype=np.float32).astype(BF)
    common = {
        "w0": w0p,
        "w1": np.asarray(W1, dtype=np.float32).astype(BF),
        "w2": np.asarray(W2, dtype=np.float32).astype(BF),
        "w3": np.asarray(W3, dtype=np.float32).astype(BF),
        "b0": np.ascontiguousarray(b0, dtype=np.float32),
        "b1": np.ascontiguousarray(b1, dtype=np.float32),
        "b2": np.ascontiguousarray(b2, dtype=np.float32),
        "b3": np.ascontiguousarray(b3, dtype=np.float32),
    }
    in_maps = []
    for c in range(N_CORES):
        in_maps.append({"xb": xb_cat[c * 128 : (c + 1) * 128], **common})
    return in_maps


def kernel(inputs, g, W0, b0, W1, b1, W2, b2, W3, b3):
    nc = _get_nc()
    in_maps = make_in_maps(inputs, W0, b0, W1, b1, W2, b2, W3, b3)
    res = run_bass_kernel_spmd(nc, in_maps, core_ids=list(range(N_CORES)))
    out = np.empty((N_ROWS, LAT), dtype=np.float32)
    for c, r in enumerate(res.results):
        out[c * ROWS_PER_CORE : (c + 1) * ROWS_PER_CORE] = r["outT"].T
    return out


# revision 4
# speedup vs baseline: 2.1135x; 1.0249x over previous
"""Trainium2 Bass kernel for a 4-layer MLP over N=100000 rows (DHGCN forward).

Reference computation (the graph edge_index `g` is dead):
    h = relu(x @ W0 + b0); h = relu(h @ W1 + b1)
    h = relu(h @ W2 + b2); h = relu(h @ W3 + b3)
with x [100000, 3000], W0 [3000,512], W1/W2 [512,512], W3 [512,20].

Strategy: data-parallel over rows across 8 NeuronCores (weights replicated).

Device-side layout/precision choices, each measured on the axon-tunneled HW:

- Matmul operands are bfloat16 (PSUM accumulation fp32, final output fp32).
  bf16 streams 1 column/cycle on the PE exactly like f32r, but its weight
  loads take the fast-weight-load path that the engine queue hoists ahead of
  in-flight matmuls, while f32r's 4-byte self-loading InstMatmult serializes
  ~128+ weight-load cycles onto every matmul (measured marginal exec 944us
  f32r vs ~690us PE streaming floor; the cost model's TODO explicitly skips
  LDWEIGHTS). bf16 also halves HBM traffic. End-to-end rel err ~5e-3 vs the
  fp32 reference (gate 2e-2).

- On host, x is cast to bf16 and permuted to [128 part, block, ktile, row] so
  each row-block DMA is 128 fully-contiguous 24KB partition reads; every
  matmul is out[M=out_feat_chunk, N=rows] = W_tile.T @ hT_tile with
  natural-layout weights and no on-device transposes.

- Row-block pairs are software-pipelined (L0(a) L0(b) L1(a) L1(b) ...): every
  matmul group has a whole other group between it and the activations that
  produce its input, so the PE never stalls on PSUM-drain + activation
  latency at layer transitions.

- ALL per-core inputs (x, weights, biases) are packed into ONE flat bf16
  DRAM tensor. The axon PJRT dispatch costs ~42us per argument per
  execution (measured by scaling a tiny kernel's input count), so going from
  10 tensors to a single blob + output removes ~350us of per-exec overhead.
"""

import numpy as np
import ml_dtypes

import concourse.bacc as bacc
import concourse.mybir as mybir
import concourse.tile as tile
from concourse.bass import ts
from concourse.bass_utils import run_bass_kernel_spmd

F32 = mybir.dt.float32
BF16 = mybir.dt.bfloat16
BF = ml_dtypes.bfloat16
RELU = mybir.ActivationFunctionType.Relu

N_CORES = 8
N_ROWS = 100000
ROWS_PER_CORE = N_ROWS // N_CORES  # 12500
R = 500                            # row-block (PSUM free dim <= 512)
N_BLK = ROWS_PER_CORE // R         # 25
IN_DIM = 3000
K0 = 3072                          # padded in_dim = 24*128
KT0 = K0 // 128                    # 24 K-tiles for layer 0
KT0_FULL = IN_DIM // 128           # 23 full K-tiles before the ragged one
KT0_REM = IN_DIM - KT0_FULL * 128  # 56 valid partitions in K-tile 23
H = 512
KT = H // 128                      # 4 K-tiles for layers 1-3
M_CH = H // 128                    # 4 output chunks of 128 for layers 0-2
LAT = 20

# Flat-blob element offsets (bf16 throughout).
X_N = 128 * N_BLK * KT0 * R        # 38_400_000
W0_N = K0 * H                      # 1_572_864
W1_N = H * H
W2_N = H * H
W3_N = H * LAT
OFF_W0 = X_N
OFF_W1 = OFF_W0 + W0_N
OFF_W2 = OFF_W1 + W1_N
OFF_W3 = OFF_W2 + W2_N
OFF_B0 = OFF_W3 + W3_N
OFF_B1 = OFF_B0 + H
OFF_B2 = OFF_B1 + H
OFF_B3 = OFF_B2 + H
BLOB_N = OFF_B3 + LAT


def build_program():
    nc = bacc.Bacc("TRN2", target_bir_lowering=False, debug=False)

    blob = nc.dram_tensor("blob", [BLOB_N], BF16, kind="ExternalInput")
    outT = nc.dram_tensor("outT", [LAT, ROWS_PER_CORE], F32, kind="ExternalOutput")

    xb = blob[:X_N].rearrange(
        "(p nb ko r) -> p nb ko r", p=128, nb=N_BLK, ko=KT0, r=R
    )
    w0r = blob[OFF_W0:OFF_W1].rearrange("(ko p f) -> p ko f", p=128, f=H)
    w1r = blob[OFF_W1:OFF_W2].rearrange("(ko p f) -> p ko f", p=128, f=H)
    w2r = blob[OFF_W2:OFF_W3].rearrange("(ko p f) -> p ko f", p=128, f=H)
    w3r = blob[OFF_W3:OFF_B0].rearrange("(ko p f) -> p ko f", p=128, f=LAT)
    b0r = blob[OFF_B0:OFF_B1].rearrange("(m p) -> p m", p=128)
    b1r = blob[OFF_B1:OFF_B2].rearrange("(m p) -> p m", p=128)
    b2r = blob[OFF_B2:OFF_B3].rearrange("(m p) -> p m", p=128)
    b3r = blob[OFF_B3:BLOB_N].rearrange("(m p) -> p m", p=LAT)

    with tile.TileContext(nc) as tc:
        with (
            tc.tile_pool(name="const", bufs=1) as const,
            tc.tile_pool(name="xin", bufs=4) as xin,
            tc.tile_pool(name="hbuf", bufs=1) as hbuf,
            tc.tile_pool(name="obuf", bufs=2) as obuf,
            tc.tile_pool(name="psA", bufs=6, space="PSUM") as psA,
            tc.tile_pool(name="psB", bufs=2, space="PSUM") as psB,
        ):
            w0_sb = const.tile([128, KT0, H], BF16, tag="w0")
            w1_sb = const.tile([128, KT, H], BF16, tag="w1")
            w2_sb = const.tile([128, KT, H], BF16, tag="w2")
            w3_sb = const.tile([128, KT, LAT], BF16, tag="w3")
            b0_sb = const.tile([128, M_CH], BF16, tag="b0")
            b1_sb = const.tile([128, M_CH], BF16, tag="b1")
            b2_sb = const.tile([128, M_CH], BF16, tag="b2")
            b3_sb = const.tile([LAT, 1], BF16, tag="b3")

            x_ts = {}

            # Startup: interleave w0 K-tile loads with block-0 x chunks so the
            # first accumulation group starts as soon as its first tiles land,
            # then stream the rest of the constants.
            x_ts[0] = xin.tile([128, KT0, R], BF16, tag="x", name="x0")
            CH = 6
            for c0 in range(0, KT0, CH):
                for ko in range(c0, c0 + CH):
                    nc.sync.dma_start(w0_sb[:, ko, :], w0r[:, ko, :])
                nc.sync.dma_start(
                    x_ts[0][:, c0 : c0 + CH, :], xb[:, 0, c0 : c0 + CH, :]
                )
            nc.sync.dma_start(b0_sb[:], b0r[:])
            x_ts[1] = xin.tile([128, KT0, R], BF16, tag="x", name="x1")
            nc.sync.dma_start(x_ts[1][:], xb[:, 1])
            nc.sync.dma_start(w1_sb[:], w1r[:])
            nc.sync.dma_start(b1_sb[:], b1r[:])
            nc.sync.dma_start(w2_sb[:], w2r[:])
            nc.sync.dma_start(b2_sb[:], b2r[:])
            nc.sync.dma_start(w3_sb[:], w3r[:])
            nc.sync.dma_start(b3_sb[:], b3r[:])

            def emit_l0(x_t, h1):
                for m in range(M_CH):
                    ps = psA.tile([128, R], F32, tag="ps", name="ps")
                    for ko in range(KT0):
                        nc.tensor.matmul(
                            ps[:],
                            w0_sb[:, ko, ts(m, 128)],
                            x_t[:, ko, :],
                            start=(ko == 0),
                            stop=(ko == KT0 - 1),
                        )
                    nc.scalar.activation(
                        h1[:, m, :], ps[:], RELU, bias=b0_sb[:, m : m + 1]
                    )

            def emit_mid(w_sb, b_sb, hin, hout):
                for m in range(M_CH):
                    ps = psA.tile([128, R], F32, tag="ps", name="ps")
                    for ko in range(KT):
                        nc.tensor.matmul(
                            ps[:],
                            w_sb[:, ko, ts(m, 128)],
                            hin[:, ko, :],
                            start=(ko == 0),
                            stop=(ko == KT - 1),
                        )
                    nc.scalar.activation(
                        hout[:, m, :], ps[:], RELU, bias=b_sb[:, m : m + 1]
                    )

            def emit_l3(j, h3):
                ps3 = psB.tile([LAT, R], F32, tag="ps3", name="ps3")
                for ko in range(KT):
                    nc.tensor.matmul(
                        ps3[:],
                        w3_sb[:, ko, :],
                        h3[:, ko, :],
                        start=(ko == 0),
                        stop=(ko == KT - 1),
                    )
                o_t = obuf.tile([LAT, R], F32, tag="o", name="o")
                nc.scalar.activation(o_t[:], ps3[:], RELU, bias=b3_sb[:])
                nc.sync.dma_start(outT[:, ts(j, R)], o_t[:])

            for a in range(0, N_BLK, 2):
                b = a + 1 if a + 1 < N_BLK else None
                h1a = hbuf.tile([128, KT, R], BF16, tag="h1a")
                emit_l0(x_ts[a], h1a)
                if b is not None:
                    h1b = hbuf.tile([128, KT, R], BF16, tag="h1b")
                    emit_l0(x_ts[b], h1b)
                # prefetch the next pair while this pair computes
                for nxt in (a + 2, a + 3):
                    if nxt < N_BLK:
                        x_ts[nxt] = xin.tile(
                            [128, KT0, R], BF16, tag="x", name=f"x{nxt}"
                        )
                        nc.sync.dma_start(x_ts[nxt][:], xb[:, nxt])
                h2a = hbuf.tile([128, KT, R], BF16, tag="h2a")
                emit_mid(w1_sb, b1_sb, h1a, h2a)
                if b is not None:
                    h2b = hbuf.tile([128, KT, R], BF16, tag="h2b")
                    emit_mid(w1_sb, b1_sb, h1b, h2b)
                h3a = hbuf.tile([128, KT, R], BF16, tag="h3a")
                emit_mid(w2_sb, b2_sb, h2a, h3a)
                if b is not None:
                    h3b = hbuf.tile([128, KT, R], BF16, tag="h3b")
                    emit_mid(w2_sb, b2_sb, h2b, h3b)
                emit_l3(a, h3a)
                if b is not None:
                    emit_l3(b, h3b)

    nc.compile()
    return nc


_NC = None


def _get_nc():
    global _NC
    if _NC is None:
        _NC = build_program()
    return _NC


def make_in_maps(inputs, W0, b0, W1, b1, W2, b2, W3, b3):
    """Host-side sharding: per core, one flat bf16 blob holding
    [x permuted to [128, N_BLK, KT0, R] with feature padding zeroed |
     w0 (padded) | w1 | w2 | w3 | b0 | b1 | b2 | b3].
    """
    x = np.asarray(inputs, dtype=np.float32)
    pack = np.empty(BLOB_N - X_N, dtype=BF)
    w0p = pack[0:W0_N].reshape(K0, H)
    w0p[:IN_DIM] = np.asarray(W0, dtype=np.float32).astype(BF)
    w0p[IN_DIM:] = 0
    pack[W0_N : W0_N + W1_N] = np.asarray(W1, np.float32).astype(BF).ravel()
    pack[W0_N + W1_N : W0_N + W1_N + W2_N] = (
        np.asarray(W2, np.float32).astype(BF).ravel()
    )
    o = W0_N + W1_N + W2_N
    pack[o : o + W3_N] = np.asarray(W3, np.float32).astype(BF).ravel()
    o += W3_N
    pack[o : o + H] = np.asarray(b0, np.float32).astype(BF)
    pack[o + H : o + 2 * H] = np.asarray(b1, np.float32).astype(BF)
    pack[o + 2 * H : o + 3 * H] = np.asarray(b2, np.float32).astype(BF)
    pack[o + 3 * H :] = np.asarray(b3, np.float32).astype(BF)

    in_maps = []
    for c in range(N_CORES):
        blob = np.empty(BLOB_N, dtype=BF)
        v = blob[:X_N].reshape(128, N_BLK, KT0, R)
        xc = x[c * ROWS_PER_CORE : (c + 1) * ROWS_PER_CORE].astype(BF)
        src = xc.reshape(N_BLK, R, IN_DIM)
        full = src[:, :, : KT0_FULL * 128].reshape(N_BLK, R, KT0_FULL, 128)
        v[:, :, :KT0_FULL, :] = full.transpose(3, 0, 2, 1)
        part = src[:, :, KT0_FULL * 128 :]  # [N_BLK, R, KT0_REM]
        v[:KT0_REM, :, KT0_FULL, :] = part.transpose(2, 0, 1)
        v[KT0_REM:, :, KT0_FULL, :] = 0
        blob[X_N:] = pack
        in_maps.append({"blob": blob})
    return in_maps


def kernel(inputs, g, W0, b0, W1, b1, W2, b2, W3, b3):
    nc = _get_nc()
    in_maps = make_in_maps(inputs, W0, b0, W1, b1, W2, b2, W3, b3)
    res = run_bass_kernel_spmd(nc, in_maps, core_ids=list(range(N_CORES)))
    out = np.empty((N_ROWS, LAT), dtype=np.float32)
    for c, r in enumerate(res.results):
        out[c * ROWS_PER_CORE : (c + 1) * ROWS_PER_CORE] = r["outT"].T
    return out


# revision 5
# speedup vs baseline: 2.1366x; 1.0109x over previous
"""Trainium2 Bass kernel for a 4-layer MLP over N=100000 rows (DHGCN forward).

Reference computation (the graph edge_index `g` is dead):
    h = relu(x @ W0 + b0); h = relu(h @ W1 + b1)
    h = relu(h @ W2 + b2); h = relu(h @ W3 + b3)
with x [100000, 3000], W0 [3000,512], W1/W2 [512,512], W3 [512,20].

Strategy: data-parallel over rows across 8 NeuronCores (weights replicated).

Device-side layout/precision choices, each measured on the axon-tunneled HW:

- Matmul operands are bfloat16 (PSUM accumulation fp32, final output fp32).
  bf16 streams 1 column/cycle on the PE exactly like f32r, but halves HBM
  traffic and avoids f32r's slow 4-byte self-loading weight path (measured
  marginal per-exec device time: 944us f32r -> 907us bf16). A PE-stream
  microbench (pebench.py) shows the device sustains only ~1.8 GHz under
  continuous dense matmul (vs the 2.4 GHz warm-clock model), and 907us is
  exactly the kernel's 1.655M streamed columns at that clock with zero
  stalls - i.e. the schedule is at the sustained-clock compute roofline.
  End-to-end rel err ~5e-3 vs the fp32 reference (gate 2e-2).

- On host, x is cast to bf16 and permuted to [128 part, block, ktile, row] so
  each row-block DMA is 128 fully-contiguous 24KB partition reads; every
  matmul is out[M=out_feat_chunk, N=rows] = W_tile.T @ hT_tile with
  natural-layout weights and no on-device transposes.

- Row-block pairs are software-pipelined (L0(a) L0(b) L1(a) L1(b) ...): every
  matmul group has a whole other group between it and the activations that
  produce its input, so the PE never stalls on PSUM-drain + activation
  latency at layer transitions.

- ALL per-core inputs (x, weights, biases) are packed into ONE flat bf16
  DRAM tensor. The axon PJRT dispatch costs ~42us per argument per
  execution (measured by scaling a tiny kernel's input count), so going from
  10 tensors to a single blob + output removes ~350us of per-exec overhead.
"""

import numpy as np
import ml_dtypes

import concourse.bacc as bacc
import concourse.mybir as mybir
import concourse.tile as tile
from concourse.bass import ts
from concourse.bass_utils import run_bass_kernel_spmd

F32 = mybir.dt.float32
BF16 = mybir.dt.bfloat16
BF = ml_dtypes.bfloat16
RELU = mybir.ActivationFunctionType.Relu

N_CORES = 8
N_ROWS = 100000
ROWS_PER_CORE = N_ROWS // N_CORES  # 12500
R = 500                            # row-block (PSUM free dim <= 512)
N_BLK = ROWS_PER_CORE // R         # 25
IN_DIM = 3000
K0 = 3072                          # padded in_dim = 24*128
KT0 = K0 // 128                    # 24 K-tiles for layer 0
KT0_FULL = IN_DIM // 128           # 23 full K-tiles before the ragged one
KT0_REM = IN_DIM - KT0_FULL * 128  # 56 valid partitions in K-tile 23
H = 512
KT = H // 128                      # 4 K-tiles for layers 1-3
M_CH = H // 128                    # 4 output chunks of 128 for layers 0-2
LAT = 20

# Flat-blob element offsets (bf16 throughout).
X_N = 128 * N_BLK * KT0 * R        # 38_400_000
W0_N = K0 * H                      # 1_572_864
W1_N = H * H
W2_N = H * H
W3_N = H * LAT
OFF_W0 = X_N
OFF_W1 = OFF_W0 + W0_N
OFF_W2 = OFF_W1 + W1_N
OFF_W3 = OFF_W2 + W2_N
OFF_B0 = OFF_W3 + W3_N
OFF_B1 = OFF_B0 + H
OFF_B2 = OFF_B1 + H
OFF_B3 = OFF_B2 + H
BLOB_N = OFF_B3 + LAT


def build_program():
    nc = bacc.Bacc("TRN2", target_bir_lowering=False, debug=False)

    blob = nc.dram_tensor("blob", [BLOB_N], BF16, kind="ExternalInput")
    outT = nc.dram_tensor("outT", [LAT, ROWS_PER_CORE], F32, kind="ExternalOutput")

    xb = blob[:X_N].rearrange(
        "(p nb ko r) -> p nb ko r", p=128, nb=N_BLK, ko=KT0, r=R
    )
    w0r = blob[OFF_W0:OFF_W1].rearrange("(ko p f) -> p ko f", p=128, f=H)
    w1r = blob[OFF_W1:OFF_W2].rearrange("(ko p f) -> p ko f", p=128, f=H)
    w2r = blob[OFF_W2:OFF_W3].rearrange("(ko p f) -> p ko f", p=128, f=H)
    w3r = blob[OFF_W3:OFF_B0].rearrange("(ko p f) -> p ko f", p=128, f=LAT)
    b0r = blob[OFF_B0:OFF_B1].rearrange("(m p) -> p m", p=128)
    b1r = blob[OFF_B1:OFF_B2].rearrange("(m p) -> p m", p=128)
    b2r = blob[OFF_B2:OFF_B3].rearrange("(m p) -> p m", p=128)
    b3r = blob[OFF_B3:BLOB_N].rearrange("(m p) -> p m", p=LAT)

    with tile.TileContext(nc) as tc:
        with (
            tc.tile_pool(name="const", bufs=1) as const,
            tc.tile_pool(name="xin", bufs=4) as xin,
            tc.tile_pool(name="hbuf", bufs=1) as hbuf,
            tc.tile_pool(name="obuf", bufs=2) as obuf,
            tc.tile_pool(name="psA", bufs=6, space="PSUM") as psA,
            tc.tile_pool(name="psB", bufs=2, space="PSUM") as psB,
        ):
            w0_sb = const.tile([128, KT0, H], BF16, tag="w0")
            w1_sb = const.tile([128, KT, H], BF16, tag="w1")
            w2_sb = const.tile([128, KT, H], BF16, tag="w2")
            w3_sb = const.tile([128, KT, LAT], BF16, tag="w3")
            b0_sb = const.tile([128, M_CH], BF16, tag="b0")
            b1_sb = const.tile([128, M_CH], BF16, tag="b1")
            b2_sb = const.tile([128, M_CH], BF16, tag="b2")
            b3_sb = const.tile([LAT, 1], BF16, tag="b3")

            x_ts = {}

            # Startup: interleave w0 K-tile loads with block-0 x chunks so the
            # first accumulation group starts as soon as its first tiles land,
            # then stream the rest of the constants.
            x_ts[0] = xin.tile([128, KT0, R], BF16, tag="x", name="x0")
            CH = 6
            for c0 in range(0, KT0, CH):
                for ko in range(c0, c0 + CH):
                    nc.sync.dma_start(w0_sb[:, ko, :], w0r[:, ko, :])
                nc.sync.dma_start(
                    x_ts[0][:, c0 : c0 + CH, :], xb[:, 0, c0 : c0 + CH, :]
                )
            nc.sync.dma_start(b0_sb[:], b0r[:])
            x_ts[1] = xin.tile([128, KT0, R], BF16, tag="x", name="x1")
            nc.sync.dma_start(x_ts[1][:], xb[:, 1])
            nc.sync.dma_start(w1_sb[:], w1r[:])
            nc.sync.dma_start(b1_sb[:], b1r[:])
            nc.sync.dma_start(w2_sb[:], w2r[:])
            nc.sync.dma_start(b2_sb[:], b2r[:])
            nc.sync.dma_start(w3_sb[:], w3r[:])
            nc.sync.dma_start(b3_sb[:], b3r[:])

            def emit_l0(x_t, h1):
                for m in range(M_CH):
                    ps = psA.tile([128, R], F32, tag="ps", name="ps")
                    for ko in range(KT0):
                        nc.tensor.matmul(
                            ps[:],
                            w0_sb[:, ko, ts(m, 128)],
                            x_t[:, ko, :],
                            start=(ko == 0),
                            stop=(ko == KT0 - 1),
                        )
                    nc.scalar.activation(
                        h1[:, m, :], ps[:], RELU, bias=b0_sb[:, m : m + 1]
                    )

            def emit_mid(w_sb, b_sb, hin, hout):
                for m in range(M_CH):
                    ps = psA.tile([128, R], F32, tag="ps", name="ps")
                    for ko in range(KT):
                        nc.tensor.matmul(
                            ps[:],
                            w_sb[:, ko, ts(m, 128)],
                            hin[:, ko, :],
                            start=(ko == 0),
                            stop=(ko == KT - 1),
                        )
                    nc.scalar.activation(
                        hout[:, m, :], ps[:], RELU, bias=b_sb[:, m : m + 1]
                    )

            def emit_l3(j, h3):
                ps3 = psB.tile([LAT, R], F32, tag="ps3", name="ps3")
                for ko in range(KT):
                    nc.tensor.matmul(
                        ps3[:],
                        w3_sb[:, ko, :],
                        h3[:, ko, :],
                        start=(ko == 0),
                        stop=(ko == KT - 1),
                    )
                o_t = obuf.tile([LAT, R], F32, tag="o", name="o")
                nc.scalar.activation(o_t[:], ps3[:], RELU, bias=b3_sb[:])
                nc.sync.dma_start(outT[:, ts(j, R)], o_t[:])

            for a in range(0, N_BLK, 2):
                b = a + 1 if a + 1 < N_BLK else None
                h1a = hbuf.tile([128, KT, R], BF16, tag="h1a")
                emit_l0(x_ts[a], h1a)
                if b is not None:
                    h1b = hbuf.tile([128, KT, R], BF16, tag="h1b")
                    emit_l0(x_ts[b], h1b)
                # prefetch the next pair while this pair computes
                for nxt in (a + 2, a + 3):
                    if nxt < N_BLK:
                        x_ts[nxt] = xin.tile(
                            [128, KT0, R], BF16, tag="x", name=f"x{nxt}"
                        )
                        nc.sync.dma_start(x_ts[nxt][:], xb[:, nxt])
                h2a = hbuf.tile([128, KT, R], BF16, tag="h2a")
                emit_mid(w1_sb, b1_sb, h1a, h2a)
                if b is not None:
                    h2b = hbuf.tile([128, KT, R], BF16, tag="h2b")
                    emit_mid(w1_sb, b1_sb, h1b, h2b)
                h3a = hbuf.tile([128, KT, R], BF16, tag="h3a")
                emit_mid(w2_sb, b2_sb, h2a, h3a)
                if b is not None:
                    h3b = hbuf.tile([128, KT, R], BF16, tag="h3b")
                    emit_mid(w2_sb, b2_sb, h2b, h3b)
                emit_l3(a, h3a)
                if b is not None:
                    emit_l3(b, h3b)

    nc.compile()
    return nc


_NC = None


def _get_nc():
    global _NC
    if _NC is None:
        _NC = build_program()
    return _NC


def make_in_maps(inputs, W0, b0, W1, b1, W2, b2, W3, b3):
    """Host-side sharding: per core, one flat bf16 blob holding
    [x permuted to [128, N_BLK, KT0, R] with feature padding zeroed |
     w0 (padded) | w1 | w2 | w3 | b0 | b1 | b2 | b3].
    """
    x = np.asarray(inputs, dtype=np.float32)
    pack = np.empty(BLOB_N - X_N, dtype=BF)
    w0p = pack[0:W0_N].reshape(K0, H)
    w0p[:IN_DIM] = np.asarray(W0, dtype=np.float32).astype(BF)
    w0p[IN_DIM:] = 0
    pack[W0_N : W0_N + W1_N] = np.asarray(W1, np.float32).astype(BF).ravel()
    pack[W0_N + W1_N : W0_N + W1_N + W2_N] = (
        np.asarray(W2, np.float32).astype(BF).ravel()
    )
    o = W0_N + W1_N + W2_N
    pack[o : o + W3_N] = np.asarray(W3, np.float32).astype(BF).ravel()
    o += W3_N
    pack[o : o + H] = np.asarray(b0, np.float32).astype(BF)
    pack[o + H : o + 2 * H] = np.asarray(b1, np.float32).astype(BF)
    pack[o + 2 * H : o + 3 * H] = np.asarray(b2, np.float32).astype(BF)
    pack[o + 3 * H :] = np.asarray(b3, np.float32).astype(BF)

    in_maps = []
    for c in range(N_CORES):
        blob = np.empty(BLOB_N, dtype=BF)
        v = blob[:X_N].reshape(128, N_BLK, KT0, R)
        xc = x[c * ROWS_PER_CORE : (c + 1) * ROWS_PER_CORE].astype(BF)
        src = xc.reshape(N_BLK, R, IN_DIM)
        full = src[:, :, : KT0_FULL * 128].reshape(N_BLK, R, KT0_FULL, 128)
        v[:, :, :KT0_FULL, :] = full.transpose(3, 0, 2, 1)
        part = src[:, :, KT0_FULL * 128 :]  # [N_BLK, R, KT0_REM]
        v[:KT0_REM, :, KT0_FULL, :] = part.transpose(2, 0, 1)
        v[KT0_REM:, :, KT0_FULL, :] = 0
        blob[X_N:] = pack
        in_maps.append({"blob": blob})
    return in_maps


def kernel(inputs, g, W0, b0, W1, b1, W2, b2, W3, b3):
    nc = _get_nc()
    in_maps = make_in_maps(inputs, W0, b0, W1, b1, W2, b2, W3, b3)
    res = run_bass_kernel_spmd(nc, in_maps, core_ids=list(range(N_CORES)))
    out = np.empty((N_ROWS, LAT), dtype=np.float32)
    for c, r in enumerate(res.results):
        out[c * ROWS_PER_CORE : (c + 1) * ROWS_PER_CORE] = r["outT"].T
    return out


# revision 12
# speedup vs baseline: 2.2539x; 1.0549x over previous
"""Trainium2 Bass kernel for a 4-layer MLP over N=100000 rows (DHGCN forward).

Reference computation (the graph edge_index `g` is dead):
    h = relu(x @ W0 + b0); h = relu(h @ W1 + b1)
    h = relu(h @ W2 + b2); h = relu(h @ W3 + b3)
with x [100000, 3000], W0 [3000,512], W1/W2 [512,512], W3 [512,20].

Strategy: data-parallel over rows across 8 NeuronCores (weights replicated).

Device-side layout/precision choices, each measured on the axon-tunneled HW:

- Matmul operands are bfloat16 (PSUM accumulation fp32, final output fp32).
  bf16 streams 1 column/cycle on the PE exactly like f32r, but halves HBM
  traffic and avoids f32r's slow 4-byte self-loading weight path (measured
  marginal per-exec device time: 944us f32r -> 907us bf16). A PE-stream
  microbench (pebench.py) shows the device sustains only ~1.8 GHz under
  continuous dense matmul (vs the 2.4 GHz warm-clock model), and 907us is
  exactly the kernel's 1.655M streamed columns at that clock with zero
  stalls - i.e. the schedule is at the sustained-clock compute roofline.
  End-to-end rel err ~5e-3 vs the fp32 reference (gate 2e-2).

- On host, x is cast to bf16 and permuted to [128 part, block, ktile, row] so
  each row-block DMA is 128 fully-contiguous 24KB partition reads; every
  matmul is out[M=out_feat_chunk, N=rows] = W_tile.T @ hT_tile with
  natural-layout weights and no on-device transposes.

- Row-block pairs are software-pipelined (L0(a) L0(b) L1(a) L1(b) ...): every
  matmul group has a whole other group between it and the activations that
  produce its input, so the PE never stalls on PSUM-drain + activation
  latency at layer transitions.

- ALL per-core inputs (x, weights, biases) are packed into ONE flat bf16
  DRAM tensor (blob + output = 2 data args instead of 10; measured ~30us/exec
  saving in axon PJRT dispatch, and it keeps host-side sharding to a single
  contiguous per-core buffer).
"""

import numpy as np
import ml_dtypes

import concourse.bacc as bacc
import concourse.mybir as mybir
import concourse.tile as tile
from concourse.bass import ts
from concourse.bass_utils import run_bass_kernel_spmd

F32 = mybir.dt.float32
BF16 = mybir.dt.bfloat16
BF = ml_dtypes.bfloat16
RELU = mybir.ActivationFunctionType.Relu

N_CORES = 8
N_ROWS = 100000
ROWS_PER_CORE = N_ROWS // N_CORES  # 12500
R = 500                            # row-block (PSUM free dim <= 512)
N_BLK = ROWS_PER_CORE // R         # 25
IN_DIM = 3000
K0 = 3072                          # padded in_dim = 24*128
KT0 = K0 // 128                    # 24 K-tiles for layer 0
KT0_FULL = IN_DIM // 128           # 23 full K-tiles before the ragged one
KT0_REM = IN_DIM - KT0_FULL * 128  # 56 valid partitions in K-tile 23
H = 512
KT = H // 128                      # 4 K-tiles for layers 1-3
M_CH = H // 128                    # 4 output chunks of 128 for layers 0-2
LAT = 20

# Flat-blob element offsets (bf16 throughout).
X_N = 128 * N_BLK * KT0 * R        # 38_400_000
W0_N = K0 * H                      # 1_572_864
W1_N = H * H
W2_N = H * H
W3_N = H * LAT
OFF_W0 = X_N
OFF_W1 = OFF_W0 + W0_N
OFF_W2 = OFF_W1 + W1_N
OFF_W3 = OFF_W2 + W2_N
OFF_B0 = OFF_W3 + W3_N
OFF_B1 = OFF_B0 + H
OFF_B2 = OFF_B1 + H
OFF_B3 = OFF_B2 + H
BLOB_N = OFF_B3 + LAT


def build_program():
    nc = bacc.Bacc("TRN2", target_bir_lowering=False, debug=False)

    blob = nc.dram_tensor("blob", [BLOB_N], BF16, kind="ExternalInput")
    outT = nc.dram_tensor("outT", [LAT, ROWS_PER_CORE], F32, kind="ExternalOutput")

    xb = blob[:X_N].rearrange(
        "(p nb ko r) -> p nb ko r", p=128, nb=N_BLK, ko=KT0, r=R
    )
    w0r = blob[OFF_W0:OFF_W1].rearrange("(ko p f) -> p ko f", p=128, f=H)
    w1r = blob[OFF_W1:OFF_W2].rearrange("(ko p f) -> p ko f", p=128, f=H)
    w2r = blob[OFF_W2:OFF_W3].rearrange("(ko p f) -> p ko f", p=128, f=H)
    w3r = blob[OFF_W3:OFF_B0].rearrange("(ko p f) -> p ko f", p=128, f=LAT)
    b0r = blob[OFF_B0:OFF_B1].rearrange("(m p) -> p m", p=128)
    b1r = blob[OFF_B1:OFF_B2].rearrange("(m p) -> p m", p=128)
    b2r = blob[OFF_B2:OFF_B3].rearrange("(m p) -> p m", p=128)
    b3r = blob[OFF_B3:BLOB_N].rearrange("(m p) -> p m", p=LAT)

    with tile.TileContext(nc) as tc:
        with (
            tc.tile_pool(name="const", bufs=1) as const,
            tc.tile_pool(name="xin", bufs=4) as xin,
            tc.tile_pool(name="hbuf", bufs=1) as hbuf,
            tc.tile_pool(name="obuf", bufs=2) as obuf,
            tc.tile_pool(name="psA", bufs=6, space="PSUM") as psA,
            tc.tile_pool(name="psB", bufs=2, space="PSUM") as psB,
        ):
            w0_sb = const.tile([128, KT0, H], BF16, tag="w0")
            w1_sb = const.tile([128, KT, H], BF16, tag="w1")
            w2_sb = const.tile([128, KT, H], BF16, tag="w2")
            w3_sb = const.tile([128, KT, LAT], BF16, tag="w3")
            b0_sb = const.tile([128, M_CH], BF16, tag="b0")
            b1_sb = const.tile([128, M_CH], BF16, tag="b1")
            b2_sb = const.tile([128, M_CH], BF16, tag="b2")
            b3_sb = const.tile([LAT, 1], BF16, tag="b3")

            x_ts = {}

            # Startup: interleave w0 K-tile loads with block-0 x chunks so the
            # first accumulation group starts as soon as its first tiles land,
            # then stream the rest of the constants.
            x_ts[0] = xin.tile([128, KT0, R], BF16, tag="x", name="x0")
            CH = 4
            for c0 in range(0, KT0, CH):
                for ko in range(c0, c0 + CH):
                    nc.sync.dma_start(w0_sb[:, ko, :], w0r[:, ko, :])
                nc.sync.dma_start(
                    x_ts[0][:, c0 : c0 + CH, :], xb[:, 0, c0 : c0 + CH, :]
                )
            nc.sync.dma_start(b0_sb[:], b0r[:])
            x_ts[1] = xin.tile([128, KT0, R], BF16, tag="x", name="x1")
            nc.sync.dma_start(x_ts[1][:], xb[:, 1])
            nc.sync.dma_start(w1_sb[:], w1r[:])
            nc.sync.dma_start(b1_sb[:], b1r[:])
            nc.sync.dma_start(w2_sb[:], w2r[:])
            nc.sync.dma_start(b2_sb[:], b2r[:])
            nc.sync.dma_start(w3_sb[:], w3r[:])
            nc.sync.dma_start(b3_sb[:], b3r[:])

            def emit_l0(x_t, h1):
                for m in range(M_CH):
                    ps = psA.tile([128, R], F32, tag="ps", name="ps")
                    for ko in range(KT0):
                        nc.tensor.matmul(
                            ps[:],
                            w0_sb[:, ko, ts(m, 128)],
                            x_t[:, ko, :],
                            start=(ko == 0),
                            stop=(ko == KT0 - 1),
                        )
                    nc.scalar.activation(
                        h1[:, m, :], ps[:], RELU, bias=b0_sb[:, m : m + 1]
                    )

            def emit_mid(w_sb, b_sb, hin, hout):
                for m in range(M_CH):
                    ps = psA.tile([128, R], F32, tag="ps", name="ps")
                    for ko in range(KT):
                        nc.tensor.matmul(
                            ps[:],
                            w_sb[:, ko, ts(m, 128)],
                            hin[:, ko, :],
                            start=(ko == 0),
                            stop=(ko == KT - 1),
                        )
                    nc.scalar.activation(
                        hout[:, m, :], ps[:], RELU, bias=b_sb[:, m : m + 1]
                    )

            def emit_l3(j, h3):
                ps3 = psB.tile([LAT, R], F32, tag="ps3", name="ps3")
                for ko in range(KT):
                    nc.tensor.matmul(
                        ps3[:],
                        w3_sb[:, ko, :],
                        h3[:, ko, :],
                        start=(ko == 0),
                        stop=(ko == KT - 1),
                    )
                o_t = obuf.tile([LAT, R], F32, tag="o", name="o")
                nc.scalar.activation(o_t[:], ps3[:], RELU, bias=b3_sb[:])
                nc.sync.dma_start(outT[:, ts(j, R)], o_t[:])

            for a in range(0, N_BLK, 2):
                b = a + 1 if a + 1 < N_BLK else None
                h1a = hbuf.tile([128, KT, R], BF16, tag="h1a")
                emit_l0(x_ts[a], h1a)
                if b is not None:
                    h1b = hbuf.tile([128, KT, R], BF16, tag="h1b")
                    emit_l0(x_ts[b], h1b)
                # prefetch the next pair while this pair computes
                for nxt in (a + 2, a + 3):
                    if nxt < N_BLK:
                        x_ts[nxt] = xin.tile(
                            [128, KT0, R], BF16, tag="x", name=f"x{nxt}"
                        )
                        nc.sync.dma_start(x_ts[nxt][:], xb[:, nxt])
                h2a = hbuf.tile([128, KT, R], BF16, tag="h2a")
                emit_mid(w1_sb, b1_sb, h1a, h2a)
                if b is not None:
                    h2b = hbuf.tile([128, KT, R], BF16, tag="h2b")
                    emit_mid(w1_sb, b1_sb, h1b, h2b)
                h3a = hbuf.tile([128, KT, R], BF16, tag="h3a")
                emit_mid(w2_sb, b2_sb, h2a, h3a)
                if b is not None:
                    h3b = hbuf.tile([128, KT, R], BF16, tag="h3b")
                    emit_mid(w2_sb, b2_sb, h2b, h3b)
                emit_l3(a, h3a)
                if b is not None:
                    emit_l3(b, h3b)

    nc.compile()
    return nc


_NC = None


def _get_nc():
    global _NC
    if _NC is None:
        _NC = build_program()
    return _NC


def make_in_maps(inputs, W0, b0, W1, b1, W2, b2, W3, b3):
    """Host-side sharding: per core, one flat bf16 blob holding
    [x permuted to [128, N_BLK, KT0, R] with feature padding zeroed |
     w0 (padded) | w1 | w2 | w3 | b0 | b1 | b2 | b3].
    """
    x = np.asarray(inputs, dtype=np.float32)
    pack = np.empty(BLOB_N - X_N, dtype=BF)
    w0p = pack[0:W0_N].reshape(K0, H)
    w0p[:IN_DIM] = np.asarray(W0, dtype=np.float32).astype(BF)
    w0p[IN_DIM:] = 0
    pack[W0_N : W0_N + W1_N] = np.asarray(W1, np.float32).astype(BF).ravel()
    pack[W0_N + W1_N : W0_N + W1_N + W2_N] = (
        np.asarray(W2, np.float32).astype(BF).ravel()
    )
    o = W0_N + W1_N + W2_N
    pack[o : o + W3_N] = np.asarray(W3, np.float32).astype(BF).ravel()
    o += W3_N
    pack[o : o + H] = np.asarray(b0, np.float32).astype(BF)
    pack[o + H : o + 2 * H] = np.asarray(b1, np.float32).astype(BF)
    pack[o + 2 * H : o + 3 * H] = np.asarray(b2, np.float32).astype(BF)
    pack[o + 3 * H :] = np.asarray(b3, np.float32).astype(BF)

    in_maps = []
    for c in range(N_CORES):
        blob = np.empty(BLOB_N, dtype=BF)
        v = blob[:X_N].reshape(128, N_BLK, KT0, R)
        xc = x[c * ROWS_PER_CORE : (c + 1) * ROWS_PER_CORE].astype(BF)
        src = xc.reshape(N_BLK, R, IN_DIM)
        full = src[:, :, : KT0_FULL * 128].reshape(N_BLK, R, KT0_FULL, 128)
        v[:, :, :KT0_FULL, :] = full.transpose(3, 0, 2, 1)
        part = src[:, :, KT0_FULL * 128 :]  # [N_BLK, R, KT0_REM]
        v[:KT0_REM, :, KT0_FULL, :] = part.transpose(2, 0, 1)
        v[KT0_REM:, :, KT0_FULL, :] = 0
        blob[X_N:] = pack
        in_maps.append({"blob": blob})
    return in_maps


def kernel(inputs, g, W0, b0, W1, b1, W2, b2, W3, b3):
    nc = _get_nc()
    in_maps = make_in_maps(inputs, W0, b0, W1, b1, W2, b2, W3, b3)
    res = run_bass_kernel_spmd(nc, in_maps, core_ids=list(range(N_CORES)))
    out = np.empty((N_ROWS, LAT), dtype=np.float32)
    for c, r in enumerate(res.results):
        out[c * ROWS_PER_CORE : (c + 1) * ROWS_PER_CORE] = r["outT"].T
    return out


# revision 14
# speedup vs baseline: 2.2725x; 1.0083x over previous
"""Trainium2 Bass kernel for a 4-layer MLP over N=100000 rows (DHGCN forward).

Reference computation (the graph edge_index `g` is dead):
    h = relu(x @ W0 + b0); h = relu(h @ W1 + b1)
    h = relu(h @ W2 + b2); h = relu(h @ W3 + b3)
with x [100000, 3000], W0 [3000,512], W1/W2 [512,512], W3 [512,20].

Strategy: data-parallel over rows across 8 NeuronCores (weights replicated).

Device-side layout/precision choices, each measured on the axon-tunneled HW:

- Matmul operands are bfloat16 (PSUM accumulation fp32, final output fp32).
  bf16 streams 1 column/cycle on the PE exactly like f32r, but halves HBM
  traffic and avoids f32r's slow 4-byte self-loading weight path (measured
  marginal per-exec device time: 944us f32r -> 907us bf16). A PE-stream
  microbench (pebench.py) shows the device sustains only ~1.8 GHz under
  continuous dense matmul (vs the 2.4 GHz warm-clock model), and 907us is
  exactly the kernel's 1.655M streamed columns at that clock with zero
  stalls - i.e. the schedule is at the sustained-clock compute roofline.
  End-to-end rel err ~5e-3 vs the fp32 reference (gate 2e-2).

- On host, x is cast to bf16 and permuted to [128 part, block, ktile, row] so
  each row-block DMA is 128 fully-contiguous 24KB partition reads; every
  matmul is out[M=out_feat_chunk, N=rows] = W_tile.T @ hT_tile with
  natural-layout weights and no on-device transposes.

- Row-block pairs are software-pipelined (L0(a) L0(b) L1(a) L1(b) ...): every
  matmul group has a whole other group between it and the activations that
  produce its input, so the PE never stalls on PSUM-drain + activation
  latency at layer transitions.

- ALL per-core inputs (x, weights, biases) are packed into ONE flat bf16
  DRAM tensor (blob + output = 2 data args instead of 10; measured ~30us/exec
  saving in axon PJRT dispatch, and it keeps host-side sharding to a single
  contiguous per-core buffer).
"""

import numpy as np
import ml_dtypes

import concourse.bacc as bacc
import concourse.mybir as mybir
import concourse.tile as tile
from concourse.bass import ts
from concourse.bass_utils import run_bass_kernel_spmd

F32 = mybir.dt.float32
BF16 = mybir.dt.bfloat16
BF = ml_dtypes.bfloat16
RELU = mybir.ActivationFunctionType.Relu

N_CORES = 8
N_ROWS = 100000
ROWS_PER_CORE = N_ROWS // N_CORES  # 12500
R = 500                            # row-block (PSUM free dim <= 512)
N_BLK = ROWS_PER_CORE // R         # 25
IN_DIM = 3000
K0 = 3072                          # padded in_dim = 24*128
KT0 = K0 // 128                    # 24 K-tiles for layer 0
KT0_FULL = IN_DIM // 128           # 23 full K-tiles before the ragged one
KT0_REM = IN_DIM - KT0_FULL * 128  # 56 valid partitions in K-tile 23
H = 512
KT = H // 128                      # 4 K-tiles for layers 1-3
M_CH = H // 128                    # 4 output chunks of 128 for layers 0-2
LAT = 20

# Flat-blob element offsets (bf16 throughout).
X_N = 128 * N_BLK * KT0 * R        # 38_400_000
W0_N = K0 * H                      # 1_572_864
W1_N = H * H
W2_N = H * H
W3_N = H * LAT
OFF_W0 = X_N
OFF_W1 = OFF_W0 + W0_N
OFF_W2 = OFF_W1 + W1_N
OFF_W3 = OFF_W2 + W2_N
OFF_B0 = OFF_W3 + W3_N
OFF_B1 = OFF_B0 + H
OFF_B2 = OFF_B1 + H
OFF_B3 = OFF_B2 + H
BLOB_N = OFF_B3 + LAT


def build_program():
    nc = bacc.Bacc("TRN2", target_bir_lowering=False, debug=False)

    blob = nc.dram_tensor("blob", [BLOB_N], BF16, kind="ExternalInput")
    outT = nc.dram_tensor("outT", [LAT, ROWS_PER_CORE], F32, kind="ExternalOutput")

    xb = blob[:X_N].rearrange(
        "(p nb ko r) -> p nb ko r", p=128, nb=N_BLK, ko=KT0, r=R
    )
    w0r = blob[OFF_W0:OFF_W1].rearrange("(ko p f) -> p ko f", p=128, f=H)
    w1r = blob[OFF_W1:OFF_W2].rearrange("(ko p f) -> p ko f", p=128, f=H)
    w2r = blob[OFF_W2:OFF_W3].rearrange("(ko p f) -> p ko f", p=128, f=H)
    w3r = blob[OFF_W3:OFF_B0].rearrange("(ko p f) -> p ko f", p=128, f=LAT)
    b0r = blob[OFF_B0:OFF_B1].rearrange("(m p) -> p m", p=128)
    b1r = blob[OFF_B1:OFF_B2].rearrange("(m p) -> p m", p=128)
    b2r = blob[OFF_B2:OFF_B3].rearrange("(m p) -> p m", p=128)
    b3r = blob[OFF_B3:BLOB_N].rearrange("(m p) -> p m", p=LAT)

    with tile.TileContext(nc) as tc:
        with (
            tc.tile_pool(name="const", bufs=1) as const,
            tc.tile_pool(name="xin", bufs=4) as xin,
            tc.tile_pool(name="hbuf", bufs=1) as hbuf,
            tc.tile_pool(name="obuf", bufs=2) as obuf,
            tc.tile_pool(name="psA", bufs=6, space="PSUM") as psA,
            tc.tile_pool(name="psB", bufs=2, space="PSUM") as psB,
        ):
            w0_sb = const.tile([128, KT0, H], BF16, tag="w0")
            w1_sb = const.tile([128, KT, H], BF16, tag="w1")
            w2_sb = const.tile([128, KT, H], BF16, tag="w2")
            w3_sb = const.tile([128, KT, LAT], BF16, tag="w3")
            b0_sb = const.tile([128, M_CH], BF16, tag="b0")
            b1_sb = const.tile([128, M_CH], BF16, tag="b1")
            b2_sb = const.tile([128, M_CH], BF16, tag="b2")
            b3_sb = const.tile([LAT, 1], BF16, tag="b3")

            x_ts = {}

            # Startup: interleave w0 K-tile loads with block-0 x chunks so the
            # first accumulation group starts as soon as its first tiles land,
            # then stream the rest of the constants.
            x_ts[0] = xin.tile([128, KT0, R], BF16, tag="x", name="x0")
            CH = 4
            for c0 in range(0, KT0, CH):
                for ko in range(c0, c0 + CH):
                    nc.sync.dma_start(w0_sb[:, ko, :], w0r[:, ko, :])
                nc.sync.dma_start(
                    x_ts[0][:, c0 : c0 + CH, :], xb[:, 0, c0 : c0 + CH, :]
                )
            nc.sync.dma_start(b0_sb[:], b0r[:])
            x_ts[1] = xin.tile([128, KT0, R], BF16, tag="x", name="x1")
            nc.sync.dma_start(x_ts[1][:], xb[:, 1])
            nc.sync.dma_start(w1_sb[:], w1r[:])
            nc.sync.dma_start(b1_sb[:], b1r[:])
            nc.sync.dma_start(w2_sb[:], w2r[:])
            nc.sync.dma_start(b2_sb[:], b2r[:])
            nc.sync.dma_start(w3_sb[:], w3r[:])
            nc.sync.dma_start(b3_sb[:], b3r[:])

            def emit_l0(x_t, h1):
                for m in range(M_CH):
                    ps = psA.tile([128, R], F32, tag="ps", name="ps")
                    for ko in range(KT0):
                        nc.tensor.matmul(
                            ps[:],
                            w0_sb[:, ko, ts(m, 128)],
                            x_t[:, ko, :],
                            start=(ko == 0),
                            stop=(ko == KT0 - 1),
                        )
                    nc.scalar.activation(
                        h1[:, m, :], ps[:], RELU, bias=b0_sb[:, m : m + 1]
                    )

            def emit_mid(w_sb, b_sb, hin, hout):
                for m in range(M_CH):
                    ps = psA.tile([128, R], F32, tag="ps", name="ps")
                    for ko in range(KT):
                        nc.tensor.matmul(
                            ps[:],
                            w_sb[:, ko, ts(m, 128)],
                            hin[:, ko, :],
                            start=(ko == 0),
                            stop=(ko == KT - 1),
                        )
                    nc.scalar.activation(
                        hout[:, m, :], ps[:], RELU, bias=b_sb[:, m : m + 1]
                    )

            def emit_l3(j, h3):
                ps3 = psB.tile([LAT, R], F32, tag="ps3", name="ps3")
                for ko in range(KT):
                    nc.tensor.matmul(
                        ps3[:],
                        w3_sb[:, ko, :],
                        h3[:, ko, :],
                        start=(ko == 0),
                        stop=(ko == KT - 1),
                    )
                o_t = obuf.tile([LAT, R], F32, tag="o", name="o")
                nc.scalar.activation(o_t[:], ps3[:], RELU, bias=b3_sb[:])
                nc.sync.dma_start(outT[:, ts(j, R)], o_t[:])

            for a in range(0, N_BLK, 2):
                b = a + 1 if a + 1 < N_BLK else None
                h1a = hbuf.tile([128, KT, R], BF16, tag="h1a")
                emit_l0(x_ts[a], h1a)
                if b is not None:
                    h1b = hbuf.tile([128, KT, R], BF16, tag="h1b")
                    emit_l0(x_ts[b], h1b)
                # prefetch the next pair while this pair computes
                for nxt in (a + 2, a + 3):
                    if nxt < N_BLK:
                        x_ts[nxt] = xin.tile(
                            [128, KT0, R], BF16, tag="x", name=f"x{nxt}"
                        )
                        nc.sync.dma_start(x_ts[nxt][:], xb[:, nxt])
                h2a = hbuf.tile([128, KT, R], BF16, tag="h2a")
                emit_mid(w1_sb, b1_sb, h1a, h2a)
                if b is not None:
                    h2b = hbuf.tile([128, KT, R], BF16, tag="h2b")
                    emit_mid(w1_sb, b1_sb, h1b, h2b)
                h3a = hbuf.tile([128, KT, R], BF16, tag="h3a")
                emit_mid(w2_sb, b2_sb, h2a, h3a)
                if b is not None:
                    h3b = hbuf.tile([128, KT, R], BF16, tag="h3b")
                    emit_mid(w2_sb, b2_sb, h2b, h3b)
                emit_l3(a, h3a)
                if b is not None:
                    emit_l3(b, h3b)

    nc.compile()
    return nc


_NC = None


def _get_nc():
    global _NC
    if _NC is None:
        _NC = build_program()
    return _NC


def make_in_maps(inputs, W0, b0, W1, b1, W2, b2, W3, b3):
    """Host-side sharding: per core, one flat bf16 blob holding
    [x permuted to [128, N_BLK, KT0, R] with feature padding zeroed |
     w0 (padded) | w1 | w2 | w3 | b0 | b1 | b2 | b3].
    """
    x = np.asarray(inputs, dtype=np.float32)
    pack = np.empty(BLOB_N - X_N, dtype=BF)
    w0p = pack[0:W0_N].reshape(K0, H)
    w0p[:IN_DIM] = np.asarray(W0, dtype=np.float32).astype(BF)
    w0p[IN_DIM:] = 0
    pack[W0_N : W0_N + W1_N] = np.asarray(W1, np.float32).astype(BF).ravel()
    pack[W0_N + W1_N : W0_N + W1_N + W2_N] = (
        np.asarray(W2, np.float32).astype(BF).ravel()
    )
    o = W0_N + W1_N + W2_N
    pack[o : o + W3_N] = np.asarray(W3, np.float32).astype(BF).ravel()
    o += W3_N
    pack[o : o + H] = np.asarray(b0, np.float32).astype(BF)
    pack[o + H : o + 2 * H] = np.asarray(b1, np.float32).astype(BF)
    pack[o + 2 * H : o + 3 * H] = np.asarray(b2, np.float32).astype(BF)
    pack[o + 3 * H :] = np.asarray(b3, np.float32).astype(BF)

    in_maps = []
    for c in range(N_CORES):
        blob = np.empty(BLOB_N, dtype=BF)
        v = blob[:X_N].reshape(128, N_BLK, KT0, R)
        xc = x[c * ROWS_PER_CORE : (c + 1) * ROWS_PER_CORE].astype(BF)
        src = xc.reshape(N_BLK, R, IN_DIM)
        full = src[:, :, : KT0_FULL * 128].reshape(N_BLK, R, KT0_FULL, 128)
        v[:, :, :KT0_FULL, :] = full.transpose(3, 0, 2, 1)
        part = src[:, :, KT0_FULL * 128 :]  # [N_BLK, R, KT0_REM]
        v[:KT0_REM, :, KT0_FULL, :] = part.transpose(2, 0, 1)
        v[KT0_REM:, :, KT0_FULL, :] = 0
        blob[X_N:] = pack
        in_maps.append({"blob": blob})
    return in_maps


def kernel(inputs, g, W0, b0, W1, b1, W2, b2, W3, b3):
    nc = _get_nc()
    in_maps = make_in_maps(inputs, W0, b0, W1, b1, W2, b2, W3, b3)
    res = run_bass_kernel_spmd(nc, in_maps, core_ids=list(range(N_CORES)))
    out = np.empty((N_ROWS, LAT), dtype=np.float32)
    for c, r in enumerate(res.results):
        out[c * ROWS_PER_CORE : (c + 1) * ROWS_PER_CORE] = r["outT"].T
    return out
